# revision 2
# baseline (speedup 1.0000x reference)
"""Trainium2 Bass kernel for nn_Atoms (8 NeuronCores, batch-parallel).

Per (b,e) pair: rfft_N -> shape mult -> irfft_N -> gaussian envelope mult ->
zero-padded rfft_2N -> phase mult -> irfft_2N[:N] -> windowed frame DFT ->
resonance scan (tensor_tensor_scan) -> inverse frame DFT -> overlap-add ->
event sum -> max_norm.  All FFTs are 4-step matmul FFTs (P=128 x Q free).
See algo.py for the validated numpy model of the same structure.
"""
import sys
sys.path.insert(0, '/opt/trn_rl_repo')
import numpy as np


def _ensure_axon_hooks():
    """Provide antenv.axon_hooks if the image lacks it (BASS_TRACE support).
    Degrades to a None hook (tracing skipped) on any failure."""
    try:
        import antenv.axon_hooks  # noqa: F401
        return
    except Exception:
        pass
    import types, contextlib, ctypes, os
    mod = types.ModuleType('antenv.axon_hooks')
    _state = {}

    def set_axon_ntff_profile_hook(h):
        _state['h'] = h

    def _make_hook():
        try:
            so = '/opt/axon/libaxon_pjrt.so'
            if not os.path.exists(so):
                return None
            lib = ctypes.CDLL(so)
            if not hasattr(lib, 'axon_start_nrt_profile'):
                return None
            lib.axon_start_nrt_profile.argtypes = [
                ctypes.POINTER(ctypes.c_int64), ctypes.c_size_t]
            lib.axon_start_nrt_profile.restype = ctypes.c_int64
            lib.axon_stop_nrt_profile.argtypes = [ctypes.c_char_p]
            lib.axon_stop_nrt_profile.restype = ctypes.c_int64

            @contextlib.contextmanager
            def _hook(output_dir, device_ids):
                import jax
                jax.devices()
                if device_ids:
                    ids = (ctypes.c_int64 * len(device_ids))(*device_ids)
                    rc = lib.axon_start_nrt_profile(ids, len(device_ids))
                else:
                    rc = lib.axon_start_nrt_profile(None, 0)
                if rc != 0:
                    raise RuntimeError(f"axon_start_nrt_profile rc={rc}")
                try:
                    yield
                finally:
                    n = lib.axon_stop_nrt_profile(str(output_dir).encode())
                    if n < 0:
                        raise RuntimeError(f"axon_stop_nrt_profile rc={n}")
            return _hook
        except Exception:
            return None

    def get_axon_ntff_profile_hook():
        if 'h' not in _state:
            _state['h'] = _make_hook()
        return _state['h']

    mod.set_axon_ntff_profile_hook = set_axon_ntff_profile_hook
    mod.get_axon_ntff_profile_hook = get_axon_ntff_profile_hook
    sys.modules['antenv.axon_hooks'] = mod
    try:
        import antenv
        antenv.axon_hooks = mod
    except Exception:
        pass


_ensure_axon_hooks()

P = 128
NS = 32768
Q1 = 256
Q2 = 512
NCB = 16385
WIN = 512
NCO = 257
NF = 128
CT = [(0, 86), (86, 172), (172, 257)]
DEBUG = None


def _wm(n, m, denom, sign, scale=1.0):
    return np.exp(sign * 2j * np.pi * np.outer(np.arange(n), np.arange(m)) / denom) * scale


def _chunk(a, rows=128):
    """[R, C] -> [nch, rows, C] zero-padded."""
    R, C = a.shape
    nch = (R + rows - 1) // rows
    out = np.zeros((nch, rows, C), a.dtype)
    for i in range(nch):
        out[i, :min(rows, R - i * rows), :] = a[i * rows:(i + 1) * rows, :]
    return out


def build_consts():
    c = {}
    s, si = -1, +1
    WPf = _wm(P, P, P, s, 1.0 / np.sqrt(NS))
    c['wpf_r'], c['wpf_i'] = WPf.real, WPf.imag
    Twf = _wm(P, Q1, NS, s)
    c['twf_r'], c['twf_i'] = Twf.real, Twf.imag
    WQf = _wm(Q1, Q1, Q1, s)                       # [n2, k2]
    c['wqf_r'], c['wqf_i'] = _chunk(WQf.real), _chunk(WQf.imag)   # [2,128,256]
    WPi = _wm(P, P, P, si, 1.0 / np.sqrt(NS))
    c['wpi_r'], c['wpi_i'] = WPi.real, WPi.imag
    Twi = _wm(P, Q1, NS, si)
    c['twi_r'], c['twi_i'] = Twi.real, Twi.imag
    WQi = _wm(Q1, Q1, Q1, si)
    c['wqi_r'], c['wqi_i'] = _chunk(WQi.real), _chunk(WQi.imag)
    WPf2 = _wm(P, P, P, s, 1.0 / np.sqrt(2 * NS))
    c['wpf2_r'], c['wpf2_i'] = WPf2.real, WPf2.imag
    Twf2 = _wm(P, Q2, 2 * NS, s)
    c['twf2_r'], c['twf2_i'] = Twf2.real, Twf2.imag
    WQf2 = _wm(Q2, NCO, Q2, s)                     # [n2, k2<=256] 512x257
    c['wqf2_r'], c['wqf2_i'] = _chunk(WQf2.real), _chunk(WQf2.imag)  # [4,128,257]
    WPi2 = _wm(P, P, P, si, 1.0 / np.sqrt(2 * NS))
    c['wpi2_r'], c['wpi2_i'] = WPi2.real, WPi2.imag
    Twi2 = _wm(P, Q2, 2 * NS, si)
    c['twi2_r'], c['twi2_i'] = Twi2.real, Twi2.imag
    c['wpi2h_r'], c['wpi2h_i'] = WPi2.real[64:128, :].copy(), WPi2.imag[64:128, :].copy()
    WQi2 = _wm(Q2, Q1, Q2, si)                     # [s2, t2<256] 512x256
    c['wqi2_r'], c['wqi2_i'] = _chunk(WQi2.real), _chunk(WQi2.imag)  # [4,128,256]
    w = np.arange(WIN)
    ham = 0.54 - 0.46 * np.cos(2.0 * np.pi * w / WIN)
    D = np.exp(-2j * np.pi * np.outer(w, np.arange(NCO)) / WIN) / np.sqrt(WIN)
    hamD = ham[:, None] * D                        # [512, 257]
    c['hd_r'], c['hd_i'] = _chunk(hamD.real), _chunk(hamD.imag)      # [4,128,257]
    coef = np.ones(NCO); coef[1:256] = 2.0
    ang = 2.0 * np.pi * np.outer(np.arange(NCO), np.arange(WIN)) / WIN
    Er = (coef[:, None] * np.cos(ang)) / np.sqrt(WIN)    # [257, 512]
    Ei = (-(coef[:, None]) * np.sin(ang)) / np.sqrt(WIN)
    c['e_r'], c['e_i'] = _chunk(Er, 86), _chunk(Ei, 86)  # [3, 86, 512]
    t = np.arange(P)[:, None] + 128.0 * np.arange(Q1)[None, :]
    c['tsq'] = t * t
    c['ident'] = np.eye(P)
    c['nident'] = -np.eye(P)
    c['jrev'] = np.eye(P)[::-1].copy()
    c['njrev'] = -np.eye(P)[::-1].copy()
    c['ones1'] = np.ones((1, P))
    return {k: np.ascontiguousarray(v, dtype=np.float32) for k, v in c.items()}


def build_pair_data(x, noise):
    B, E = x.shape[:2]
    x = np.clip(x.astype(np.float64), 0.0, 1.0)
    means = x[..., 0]; stds = x[..., 1]
    res = 0.01 + 0.99 * x[..., 2:259]
    spec_shape = x[..., 259:-1]
    amps = x[..., -1]
    sigma = np.clip((1e-8 + stds) * NS, 0.0, NS - 1.0)
    d = {}
    pos = np.clip((np.arange(NCB) + 0.5) * (128.0 / NCB) - 0.5, 0.0, 127.0)
    i0 = np.floor(pos).astype(int); i1 = np.minimum(i0 + 1, 127); wgt = pos - i0
    shp = spec_shape[..., i0] * (1.0 - wgt) + spec_shape[..., i1] * wgt
    full = np.zeros((B, E, NS))
    full[..., :NCB] = shp
    full[..., NCB:] = shp[..., 1:NCB - 1][..., ::-1]
    d['ginv'] = full.reshape(B, E, P, Q1)
    c1 = -0.5 / (sigma * sigma)
    corr = 1.0 / (1.0 + 1e-8 * sigma * np.sqrt(2.0 * np.pi))
    p3 = np.stack([np.repeat(c1[..., None], P, -1),
                   np.repeat(np.log(corr)[..., None], P, -1),
                   np.repeat(amps[..., None], P, -1)], axis=-1)   # [B,E,128,3]
    d['p3'] = p3
    theta = 2.0 * np.pi * (means * 32768.0) / 32769.0
    u = np.exp(-1j * theta[..., None] * np.arange(P))
    v = np.exp(-1j * theta[..., None] * 128.0 * np.arange(NCO))
    uv = np.concatenate([
        np.stack([u.real, u.imag], axis=-2),                      # [B,E,2,128]
        np.stack([v.real, -v.imag], axis=-2),                     # [B,E,2,257]
        np.stack([v.imag, v.real], axis=-2)], axis=-1)            # -> [B,E,2,642]
    d['uv'] = uv
    r3 = np.zeros(res.shape[:-1] + (86, 3))
    r3[..., 0:86, 0] = res[..., 0:86]
    r3[..., 0:86, 1] = res[..., 86:172]
    r3[..., 0:85, 2] = res[..., 172:257]
    d['res3'] = r3
    d['noise'] = noise
    return {k: np.ascontiguousarray(v, dtype=np.float32) for k, v in d.items()}


def build_program(n_batch, n_event):
    import concourse.bass as bass
    import concourse.mybir as mybir
    from concourse.tile import TileContext as TileContextSplitDrain

    def split_excess_waits(nc_, max_waits=1):
        # this container's walrus rejects instructions with >2 sync waits;
        # hoist excess waits onto same-engine NoOps inserted before them.
        n_split = 0
        for f in nc_.m.functions:
            for bb in f.blocks:
                out = []
                for inst in bb.instructions:
                    si = inst.sync_info
                    waits = list(si.on_wait) if si is not None else []
                    if len(waits) > max_waits:
                        head, rest = waits[:max_waits], waits[max_waits:]
                        k = 0
                        while rest:
                            nop = mybir.InstNoOp(name=f"{inst.name}-w{k}", ins=[], outs=[])
                            nop.engine = inst.engine
                            nop.sync_info = mybir.SyncInfo(on_wait=rest[:max_waits],
                                                           on_update=[])
                            out.append(nop)
                            rest = rest[max_waits:]
                            k += 1
                        inst.sync_info = mybir.SyncInfo(on_wait=head,
                                                        on_update=list(si.on_update))
                        n_split += 1
                    out.append(inst)
                bb.instructions = out
        return n_split
    f32 = mybir.dt.float32
    AT = mybir.ActivationFunctionType
    OP = mybir.AluOpType
    nc = bass.Bass()

    CN = build_consts()
    dt = {k: nc.dram_tensor(f"c_{k}", list(v.shape), f32, kind="ExternalInput")
          for k, v in CN.items()}
    din = {}
    npair = n_batch * n_event
    shapes = {'noise': [npair, P, Q1], 'ginv': [npair, P, Q1],
              'p3': [npair, P, 3], 'uv': [npair, 2, 642],
              'res3': [npair, 86, 3]}
    for k, sshape in shapes.items():
        din[k] = nc.dram_tensor(k, sshape, f32, kind="ExternalInput")
    out_d = nc.dram_tensor("out", [n_batch, P, Q1], f32, kind="ExternalOutput")
    dbg_d = nc.dram_tensor("dbg", [6, P, Q2], f32, kind="ExternalOutput")

    with TileContextSplitDrain(nc) as tc:
        with tc.tile_pool(name="const", bufs=1) as cp, \
             tc.tile_pool(name="work", bufs=1) as wp, \
             tc.tile_pool(name="acc", bufs=1) as accp, \
             tc.tile_pool(name="pre", bufs=1) as prep, \
             tc.tile_pool(name="ps", bufs=1, space="PSUM") as pp, \
             tc.tile_pool(name="pst", bufs=2, space="PSUM") as pt_pool:
            ct = {}
            for k, v in CN.items():
                if v.ndim == 3:   # chunked: tile [rows, nch*cols]
                    nch, rows, cols = v.shape
                    t = cp.tile([rows, nch * cols], f32, name=f"c_{k}", tag=f"c_{k}")
                    for i in range(nch):
                        nc.sync.dma_start(t[:, i * cols:(i + 1) * cols], dt[k][i, :, :])
                else:
                    t = cp.tile(list(v.shape), f32, tag=f"c_{k}")
                    nc.sync.dma_start(t[:], dt[k][:])
                ct[k] = t

            def chunk_ap(name, i, cols):
                return ct[name][:, i * cols:(i + 1) * cols]

            def dbg_tap(stage, *aps):
                if DEBUG == stage:
                    for i, ap in enumerate(aps):
                        pp_, ff_ = ap.partition_size(), ap.free_size()
                        nc.sync.dma_start(dbg_d[i, 0:pp_, 0:ff_], ap)

            sig_tot = accp.tile([P, n_batch * Q1], f32, name="sigtot", tag="sigtot")

            def cmul(out_r, out_i, ar, ai, br, bi, tag, fdim):
                t1 = wp.tile([P, fdim], f32, name=tag + "1", tag=tag + "1")
                t2 = wp.tile([P, fdim], f32, name=tag + "2", tag=tag + "2")
                nc.vector.tensor_mul(t1[:], ar, br)
                nc.vector.tensor_mul(t2[:], ai, bi)
                nc.vector.tensor_sub(out_r, t1[:], t2[:])
                nc.vector.tensor_mul(t1[:], ar, bi)
                nc.vector.tensor_mul(t2[:], ai, br)
                nc.vector.tensor_add(out_i, t1[:], t2[:])

            def tr(out_psum, in_sbuf, neg=False, ksize=P, iname=None):
                if iname is None:
                    iname = 'nident' if neg else 'ident'
                elif neg:
                    iname = 'n' + iname
                nc.tensor.transpose(out_psum, in_sbuf, ct[iname][0:ksize, 0:ksize])

            def fft_stage2(btr, bti, nti, wr_name, wi_name, nch, cols, out_r, out_i=None):
                """X = (btr + i bti)^T-chunks @ (Wr + i Wi); nti = -bti."""
                for ch in range(nch):
                    cs = slice(ch * P, (ch + 1) * P)
                    nc.tensor.matmul(out_r, btr[:, cs], chunk_ap(wr_name, ch, cols),
                                     start=(ch == 0), stop=False)
                    if out_i is not None:
                        nc.tensor.matmul(out_i, btr[:, cs], chunk_ap(wi_name, ch, cols),
                                         start=(ch == 0), stop=False)
                for ch in range(nch):
                    cs = slice(ch * P, (ch + 1) * P)
                    nc.tensor.matmul(out_r, nti[:, cs], chunk_ap(wi_name, ch, cols),
                                     start=False, stop=(ch == nch - 1))
                    if out_i is not None:
                        nc.tensor.matmul(out_i, bti[:, cs], chunk_ap(wr_name, ch, cols),
                                         start=False, stop=(ch == nch - 1))

            def transpose_to_chunks(src_r, src_i, nch, tag):
                """[128, nch*128] -> transposed chunks [n2c, k1] side by side."""
                otr = wp.tile([P, nch * P], f32, name=tag + "r", tag=tag + "r")
                oti = wp.tile([P, nch * P], f32, name=tag + "i", tag=tag + "i")
                for ch in range(nch):
                    cs = slice(ch * P, (ch + 1) * P)
                    tp = pt_pool.tile([P, P], f32, name="T", tag="T")
                    tr(tp[:], src_r[:, cs])
                    nc.scalar.copy(otr[:, cs], tp[:])
                    tp2 = pt_pool.tile([P, P], f32, name="T", tag="T")
                    tr(tp2[:], src_i[:, cs])
                    nc.scalar.copy(oti[:, cs], tp2[:])
                nti = wp.tile([P, nch * P], f32, name=tag + "n", tag=tag + "n")
                nc.scalar.mul(nti[:], oti[:], -1.0)
                return otr, oti, nti

            if True:
                nc.vector.memset(sig_tot[:], 0.0)
                with tc.For_i(0, npair, 1) as ev:
                    xg = wp.tile([P, Q1], f32, name="xg", tag="xg")
                    nc.sync.dma_start(xg[:], din['noise'][ev, :, :])
                    xg = xg[:]

                    # ---------- forward FFT_N ----------
                    ps_a = pp.tile([P, Q2], f32, name="A", tag="A")
                    ps_b = pp.tile([P, Q2], f32, name="B", tag="B")
                    nc.tensor.matmul(ps_a[:, 0:Q1], ct['wpf_r'][:], xg, start=True, stop=True)
                    nc.tensor.matmul(ps_b[:, 0:Q1], ct['wpf_i'][:], xg, start=True, stop=True)
                    bpr = wp.tile([P, Q1], f32, name="bpr", tag="bpr")
                    bpi = wp.tile([P, Q1], f32, name="bpi", tag="bpi")
                    cmul(bpr[:], bpi[:], ps_a[:, 0:Q1], ps_b[:, 0:Q1],
                         ct['twf_r'][:], ct['twf_i'][:], "tA", Q1)
                    btr, bti, nti = transpose_to_chunks(bpr, bpi, 2, "bt")
                    ps_c = pp.tile([P, Q2], f32, name="C", tag="A")
                    ps_d = pp.tile([P, Q2], f32, name="D", tag="B")
                    fft_stage2(btr, bti, nti, 'wqf_r', 'wqf_i', 2, Q1,
                               ps_c[:, 0:Q1], ps_d[:, 0:Q1])
                    spr = wp.tile([P, Q1], f32, name="spr", tag="spr")
                    spi = wp.tile([P, Q1], f32, name="spi", tag="spi")
                    nc.scalar.copy(spr[:], ps_c[:, 0:Q1])
                    nc.scalar.copy(spi[:], ps_d[:, 0:Q1])
                    dbg_tap('sp', spr[:], spi[:])

                    # ---------- shape mult + inverse layout ----------
                    ginv = wp.tile([P, Q1], f32, name="ginv", tag="ginv")
                    nc.sync.dma_start(ginv[:], din['ginv'][ev, :, :])
                    ginv = ginv[:]
                    inv_r = wp.tile([P, Q1], f32, name="inv_r", tag="inv_r")
                    inv_i = wp.tile([P, Q1], f32, name="inv_i", tag="inv_i")
                    for src, dst in [(spr, inv_r), (spi, inv_i)]:
                        for half in range(2):
                            tp = pt_pool.tile([P, P], f32, name="T", tag="T")
                            tr(tp[:], src[:, half::2])
                            nc.vector.tensor_mul(dst[:, half * P:(half + 1) * P],
                                                 tp[:], ginv[:, half * P:(half + 1) * P])

                    dbg_tap('inv', inv_r[:], inv_i[:])
                    # ---------- inverse FFT_N -> band_noise (y-grid) ----------
                    ps_a = pp.tile([P, Q2], f32, name="A", tag="A")
                    ps_b = pp.tile([P, Q2], f32, name="B", tag="B")
                    nii = wp.tile([P, Q1], f32, name="nii", tag="nii")
                    nc.scalar.mul(nii[:], inv_i[:], -1.0)
                    nc.tensor.matmul(ps_a[:, 0:Q1], ct['wpi_r'][:], inv_r[:], start=True, stop=False)
                    nc.tensor.matmul(ps_a[:, 0:Q1], ct['wpi_i'][:], nii[:], start=False, stop=True)
                    nc.tensor.matmul(ps_b[:, 0:Q1], ct['wpi_i'][:], inv_r[:], start=True, stop=False)
                    nc.tensor.matmul(ps_b[:, 0:Q1], ct['wpi_r'][:], inv_i[:], start=False, stop=True)
                    cpr = wp.tile([P, Q1], f32, name="bpr", tag="bpr")
                    cpi = wp.tile([P, Q1], f32, name="bpi", tag="bpi")
                    cmul(cpr[:], cpi[:], ps_a[:, 0:Q1], ps_b[:, 0:Q1],
                         ct['twi_r'][:], ct['twi_i'][:], "tA", Q1)
                    ctr, cti, ncti = transpose_to_chunks(cpr, cpi, 2, "bt")
                    ps_c = pp.tile([P, Q2], f32, name="C", tag="A")
                    fft_stage2(ctr, cti, ncti, 'wqi_r', 'wqi_i', 2, Q1, ps_c[:, 0:Q1])

                    # ---------- probs * band_noise -> a_y ----------
                    p3t = wp.tile([P, 3], f32, name="p3t", tag="p3t")
                    nc.sync.dma_start(p3t[:], din['p3'][ev, :, :])
                    probs = wp.tile([P, Q1], f32, name="probs", tag="probs")
                    nc.scalar.activation(probs[:], ct['tsq'][:], AT.Exp,
                                         bias=p3t[:, 1:2], scale=p3t[:, 0:1])
                    a_y = wp.tile([P, Q1], f32, name="a_y", tag="a_y")
                    nc.vector.tensor_mul(a_y[:], probs[:], ps_c[:, 0:Q1])
                    dbg_tap('ay', a_y[:], probs[:])

                    # ---------- remap a_y -> a2 [64, 512] ----------
                    a2 = wp.tile([64, Q2], f32, name="a2", tag="a2")
                    for q in range(4):
                        tp = pt_pool.tile([P, P], f32, name="T", tag="T")
                        tr(tp[0:64, :], a_y[:, q::4])
                        nc.scalar.copy(a2[:, q * P:(q + 1) * P], tp[0:64, :])

                    dbg_tap('a2', a2[:])
                    # ---------- forward FFT_2N ----------
                    ps_a = pp.tile([P, Q2], f32, name="A", tag="A")
                    ps_b = pp.tile([P, Q2], f32, name="B", tag="B")
                    nc.tensor.matmul(ps_a[:], ct['wpf2_r'][0:64, :], a2[:], start=True, stop=True)
                    nc.tensor.matmul(ps_b[:], ct['wpf2_i'][0:64, :], a2[:], start=True, stop=True)
                    dpr = wp.tile([P, Q2], f32, name="dpr", tag="dpr")
                    dpi = wp.tile([P, Q2], f32, name="dpi", tag="dpi")
                    cmul(dpr[:], dpi[:], ps_a[:], ps_b[:],
                         ct['twf2_r'][:], ct['twf2_i'][:], "tB", Q2)
                    dtr, dti, ndti = transpose_to_chunks(dpr, dpi, 4, "dt")
                    ps_c = pp.tile([P, Q2], f32, name="C", tag="A")
                    ps_d = pp.tile([P, Q2], f32, name="D", tag="B")
                    fft_stage2(dtr, dti, ndti, 'wqf2_r', 'wqf2_i', 4, NCO,
                               ps_c[:, 0:NCO], ps_d[:, 0:NCO])
                    s2r = wp.tile([P, NCO], f32, name="s2r", tag="s2r")
                    s2i = wp.tile([P, NCO], f32, name="s2i", tag="s2i")
                    nc.scalar.copy(s2r[:], ps_c[:, 0:NCO])
                    nc.scalar.copy(s2i[:], ps_d[:, 0:NCO])
                    dbg_tap('s2', s2r[:], s2i[:])

                    # ---------- phase multiply ----------
                    uvt = wp.tile([2, 642], f32, name="uvt", tag="uvt")
                    nc.sync.dma_start(uvt[:], din['uv'][ev, :, :])
                    ps_a = pp.tile([P, Q2], f32, name="A", tag="A")
                    ps_b = pp.tile([P, Q2], f32, name="B", tag="B")
                    nc.tensor.matmul(ps_a[:, 0:NCO], uvt[:, 0:P], uvt[:, P:P + NCO],
                                     start=True, stop=True)
                    nc.tensor.matmul(ps_b[:, 0:NCO], uvt[:, 0:P], uvt[:, P + NCO:642],
                                     start=True, stop=True)
                    y_r = wp.tile([P, NCO], f32, name="y_r", tag="y_r")
                    y_i = wp.tile([P, NCO], f32, name="y_i", tag="y_i")
                    cmul(y_r[:], y_i[:], s2r[:], s2i[:],
                         ps_a[:, 0:NCO], ps_b[:, 0:NCO], "tC", NCO)
                    dbg_tap('y', y_r[:], y_i[:])

                    # ---------- hermitian extension: lo rows 0..63, hi rows 64..127 ----------
                    inv2_r = wp.tile([64, Q2], f32, name="inv2_r", tag="inv2_r")
                    inv2_i = wp.tile([64, Q2], f32, name="inv2_i", tag="inv2_i")
                    inv2h_r = wp.tile([64, Q2], f32, name="inv2h_r", tag="inv2h_r")
                    inv2h_i = wp.tile([64, Q2], f32, name="inv2h_i", tag="inv2h_i")
                    yrev_r = wp.tile([P, NCO], f32, name="yrev_r", tag="yrev_r")
                    yrev_i = wp.tile([P, NCO], f32, name="yrev_i", tag="yrev_i")
                    nc.vector.tensor_copy(yrev_r[:], y_r[:, 256::-1])
                    nc.vector.tensor_copy(yrev_i[:], y_i[:, 256::-1])
                    # region A rows 0..63: Inv2[s1, 128q+lo] = Y[lo, 4 s1 + q]
                    for q in range(4):
                        for yy, dst in [(y_r, inv2_r), (y_i, inv2_i)]:
                            tp = pt_pool.tile([P, P], f32, name="T", tag="T")
                            tr(tp[0:64, :], yy[:, q::4][:, 0:64])
                            nc.scalar.copy(dst[:, q * P:(q + 1) * P], tp[0:64, :])
                    # hi rows (global 64+a): col 0: conj(Y[0, 256-4a]) = conj(yrev[0,4a])
                    for yy, dst, ng in [(yrev_r, inv2h_r, False), (yrev_i, inv2h_i, True)]:
                        tp = pt_pool.tile([P, P], f32, name="T", tag="T")
                        tr(tp[0:64, 0:1], yy[0:1, 0:256:4], ksize=1)
                        if ng:
                            nc.scalar.mul(dst[:, 0:1], tp[0:64, 0:1], -1.0)
                        else:
                            nc.scalar.copy(dst[:, 0:1], tp[0:64, 0:1])
                    # ...except global row 64 col 0 = direct Y[0, 256]
                    nc.scalar.copy(inv2h_r[0:1, 0:1], y_r[0:1, 256:257])
                    nc.scalar.copy(inv2h_i[0:1, 0:1], y_i[0:1, 256:257])
                    # region B: hi[a, 128*blk+mu] = conj(Y[128-mu, 252+qp-4a]) via J-transpose
                    for qp in range(4):
                        blk = 3 - qp
                        for yy, dst, ng in [(yrev_r, inv2h_r, False), (yrev_i, inv2h_i, True)]:
                            tp = pt_pool.tile([P, P], f32, name="T", tag="T")
                            tr(tp[0:64, :], yy[:, 4 - qp::4][:, 0:64], iname='jrev')
                            if ng:
                                nc.scalar.mul(dst[:, blk * P + 1:blk * P + 128],
                                              tp[0:64, 0:127], -1.0)
                            else:
                                nc.scalar.copy(dst[:, blk * P + 1:blk * P + 128],
                                               tp[0:64, 0:127])
                        if qp >= 1:
                            # hi[a, (4-qp)*128] = conj(Y[0, 252+qp-4a]) = conj(yrev[0, 4-qp+4a])
                            for yy, dst, ng in [(yrev_r, inv2h_r, False), (yrev_i, inv2h_i, True)]:
                                tp = pt_pool.tile([P, P], f32, name="T", tag="T")
                                tr(tp[0:64, 0:1], yy[0:1, 4 - qp::4][:, 0:64], ksize=1)
                                if ng:
                                    nc.scalar.mul(dst[:, (4 - qp) * P:(4 - qp) * P + 1],
                                                  tp[0:64, 0:1], -1.0)
                                else:
                                    nc.scalar.copy(dst[:, (4 - qp) * P:(4 - qp) * P + 1],
                                                   tp[0:64, 0:1])

                    dbg_tap('inv2', inv2_r[:], inv2_i[:], inv2h_r[:], inv2h_i[:])
                    # ---------- inverse FFT_2N -> atoms (y-grid, real) ----------
                    ps_a = pp.tile([P, Q2], f32, name="A", tag="A")
                    ps_b = pp.tile([P, Q2], f32, name="B", tag="B")
                    ni2 = wp.tile([64, Q2], f32, name="ni2", tag="ni2")
                    ni2h = wp.tile([64, Q2], f32, name="ni2h", tag="ni2h")
                    nc.scalar.mul(ni2[:], inv2_i[:], -1.0)
                    nc.scalar.mul(ni2h[:], inv2h_i[:], -1.0)
                    nc.tensor.matmul(ps_a[:], ct['wpi2_r'][0:64, :], inv2_r[:], start=True, stop=False)
                    nc.tensor.matmul(ps_a[:], ct['wpi2h_r'][:], inv2h_r[:], start=False, stop=False)
                    nc.tensor.matmul(ps_a[:], ct['wpi2_i'][0:64, :], ni2[:], start=False, stop=False)
                    nc.tensor.matmul(ps_a[:], ct['wpi2h_i'][:], ni2h[:], start=False, stop=True)
                    nc.tensor.matmul(ps_b[:], ct['wpi2_i'][0:64, :], inv2_r[:], start=True, stop=False)
                    nc.tensor.matmul(ps_b[:], ct['wpi2h_i'][:], inv2h_r[:], start=False, stop=False)
                    nc.tensor.matmul(ps_b[:], ct['wpi2_r'][0:64, :], inv2_i[:], start=False, stop=False)
                    nc.tensor.matmul(ps_b[:], ct['wpi2h_r'][:], inv2h_i[:], start=False, stop=True)
                    epr = wp.tile([P, Q2], f32, name="dpr", tag="dpr")
                    epi = wp.tile([P, Q2], f32, name="dpi", tag="dpi")
                    cmul(epr[:], epi[:], ps_a[:], ps_b[:],
                         ct['twi2_r'][:], ct['twi2_i'][:], "tB", Q2)
                    etr, eti, neti = transpose_to_chunks(epr, epi, 4, "dt")
                    ps_c = pp.tile([P, Q2], f32, name="C", tag="A")
                    fft_stage2(etr, eti, neti, 'wqi2_r', 'wqi2_i', 4, Q1, ps_c[:, 0:Q1])

                    # ---------- atoms * amps -> SBUF (padded +2 cols) ----------
                    atoms = wp.tile([P, Q1 + 2], f32, name="atoms", tag="atoms")
                    nc.vector.memset(atoms[:, Q1:Q1 + 2], 0.0)
                    nc.scalar.activation(atoms[:, 0:Q1], ps_c[:, 0:Q1], AT.Copy,
                                         scale=p3t[:, 2:3])

                    dbg_tap('atoms', atoms[:])
                    # ---------- frame DFT + scan + inverse frame DFT ----------
                    rest = wp.tile([86, 3], f32, name="rest", tag="rest")
                    nc.sync.dma_start(rest[:], din['res3'][ev, :, :])
                    fin_r = [wp.tile([hi - lo, NF], f32, name=f"finr{i}", tag=f"finr{i}") for i, (lo, hi) in enumerate(CT)]
                    fin_i = [wp.tile([hi - lo, NF], f32, name=f"fini{i}", tag=f"fini{i}") for i, (lo, hi) in enumerate(CT)]
                    for i, (lo, hi) in enumerate(CT):
                        n_c = hi - lo
                        sp_r = pt_pool.tile([P, NF], f32, name="S", tag="S")
                        sp_i = pt_pool.tile([P, NF], f32, name="S", tag="S")
                        for u in range(4):
                            rhs = atoms[:, u:min(u + 256, 258):2]
                            nc.tensor.matmul(sp_r[0:n_c, :],
                                             chunk_ap('hd_r', u, NCO)[:, lo:hi], rhs,
                                             start=(u == 0), stop=(u == 3))
                            nc.tensor.matmul(sp_i[0:n_c, :],
                                             chunk_ap('hd_i', u, NCO)[:, lo:hi], rhs,
                                             start=(u == 0), stop=(u == 3))
                        rb = wp.tile([P, NF], f32, name="rb", tag="rb")
                        nc.scalar.activation(rb[0:n_c, :], ct['tsq'][0:n_c, 0:NF],
                                             AT.Identity, bias=rest[0:n_c, i:i + 1], scale=0.0)
                        nc.vector.tensor_tensor_scan(fin_r[i][:], rb[0:n_c, :], sp_r[0:n_c, :],
                                                     initial=sp_r[0:n_c, 0:1],
                                                     op0=mybir.AluOpType.mult,
                                                     op1=mybir.AluOpType.add)
                        nc.vector.tensor_tensor_scan(fin_i[i][:], rb[0:n_c, :], sp_i[0:n_c, :],
                                                     initial=sp_i[0:n_c, 0:1],
                                                     op0=mybir.AluOpType.mult,
                                                     op1=mybir.AluOpType.add)

                    sg = wp.tile([P, Q1], f32, name="sg", tag="sg")
                    for u in range(4):
                        of = pt_pool.tile([P, NF], f32, name="O", tag="O")
                        ws = slice(u * P, (u + 1) * P)
                        for i, (lo, hi) in enumerate(CT):
                            nc.tensor.matmul(of[:], ct['e_r'][0:hi - lo, ws] if False else
                                             chunk_ap('e_r', i, Q2)[0:hi - lo, ws],
                                             fin_r[i][:], start=(i == 0), stop=False)
                            nc.tensor.matmul(of[:], chunk_ap('e_i', i, Q2)[0:hi - lo, ws],
                                             fin_i[i][:], start=False, stop=(i == 2))
                        # ---------- OLA ----------
                        if u < 2:
                            nc.scalar.copy(sg[:, u::2], of[:])
                        else:
                            nc.vector.tensor_add(sg[:, u::2], sg[:, u::2], of[:, 0:127])
                    dbg_tap('sg', sg[:])
                    dbg_tap('fin', *[f[:] for f in fin_r[:3]], *[f[:] for f in fin_i[:3]])
                    sslice = sig_tot[:, bass.ds((ev // n_event) * Q1, Q1)]
                    nc.vector.tensor_add(sslice, sslice, sg[:])

            for b in range(n_batch):
                # ---------- max_norm ----------
                sb_ = sig_tot[:, b * Q1:(b + 1) * Q1]
                mx = wp.tile([P, 1], f32, name="mx", tag="mx")
                nc.vector.tensor_reduce(mx[:], sb_, axis=mybir.AxisListType.X,
                                        op=mybir.AluOpType.max, apply_absolute_value=True)
                tpm = pt_pool.tile([P, P], f32, name="T", tag="T")
                tr(tpm[0:1, :], mx[:])
                mxs = wp.tile([1, P], f32, name="mxs", tag="mxs")
                nc.scalar.copy(mxs[:], tpm[0:1, :])
                m11 = wp.tile([1, 1], f32, name="m11", tag="m11")
                nc.vector.tensor_reduce(m11[:], mxs[:], axis=mybir.AxisListType.X,
                                        op=mybir.AluOpType.max)
                bc = pt_pool.tile([P, P], f32, name="T", tag="T")
                nc.tensor.matmul(bc[:, 0:1], ct['ones1'][:], m11[:], start=True, stop=True)
                bcs = wp.tile([P, 1], f32, name="bcs", tag="bcs")
                nc.vector.tensor_scalar_add(bcs[:], bc[:, 0:1], 1e-8)
                rcp = wp.tile([P, 1], f32, name="rcp", tag="rcp")
                nc.vector.reciprocal(rcp[:], bcs[:])
                outt = wp.tile([P, Q1], f32, name="outt", tag="outt")
                nc.scalar.activation(outt[:], sb_, AT.Copy, scale=rcp[:])
                nc.sync.dma_start(out_d[b, :, :], outt[:])

    split_excess_waits(nc)
    return nc, CN


def kernel(x, noise):
    from concourse.bass_utils import run_bass_kernel_spmd
    x = np.asarray(x, dtype=np.float32)
    noise = np.asarray(noise, dtype=np.float32)
    B, E = x.shape[:2]
    n_cores = 8
    nb = B // n_cores
    nc, CN = build_program(nb, E)
    pd = build_pair_data(x, noise)
    in_maps = []
    for c in range(n_cores):
        m = {f"c_{k}": v for k, v in CN.items()}
        sl = slice(c * nb, (c + 1) * nb)
        m['noise'] = np.ascontiguousarray(
            pd['noise'][sl].reshape(nb * E, P, Q1))
        for k in ['ginv', 'p3', 'uv', 'res3']:
            v = pd[k][sl]
            m[k] = np.ascontiguousarray(v.reshape(nb * E, *v.shape[2:]))
        in_maps.append(m)
    res = run_bass_kernel_spmd(nc, in_maps, core_ids=list(range(n_cores)))
    global LAST_RESULT
    LAST_RESULT = res
    out = np.zeros((B, 1, NS), dtype=np.float32)
    for c in range(n_cores):
        o = res.results[c]['out']
        for bb in range(nb):
            out[c * nb + bb, 0, :] = o[bb].T.reshape(-1)
    return out



# revision 31
# speedup vs baseline: 3.6629x; 3.6629x over previous
"""Trainium2 Bass kernel for nn_Atoms (8 NeuronCores, batch-parallel). v2.

Per (b,e) pair: rfft_N -> shape mult -> irfft_N -> gaussian envelope mult ->
zero-padded rfft_2N -> phase mult -> irfft_2N[:N] -> windowed frame DFT ->
resonance scan -> inverse frame DFT -> overlap-add -> event sum -> max_norm.
All FFTs are 4-step matmul FFTs (P=128 x Q free), all matmul operands fp16.
Pairs are processed in groups of 2 (shared matmuls, N=512), 4 groups per
For_i body; twiddle multiplies run on transposed data so they double as the
transpose-PSUM drain.
"""
import sys
sys.path.insert(0, '/opt/trn_rl_repo')
import numpy as np

F16 = np.float16


def _ensure_axon_hooks():
    """Provide antenv.axon_hooks if the image lacks it (BASS_TRACE support).
    Degrades to a None hook (tracing skipped) on any failure."""
    try:
        import antenv.axon_hooks  # noqa: F401
        return
    except Exception:
        pass
    import types, contextlib, ctypes, os
    mod = types.ModuleType('antenv.axon_hooks')
    _state = {}

    def set_axon_ntff_profile_hook(h):
        _state['h'] = h

    def _make_hook():
        try:
            so = '/opt/axon/libaxon_pjrt.so'
            if not os.path.exists(so):
                return None
            lib = ctypes.CDLL(so)
            if not hasattr(lib, 'axon_start_nrt_profile'):
                return None
            lib.axon_start_nrt_profile.argtypes = [
                ctypes.POINTER(ctypes.c_int64), ctypes.c_size_t]
            lib.axon_start_nrt_profile.restype = ctypes.c_int64
            lib.axon_stop_nrt_profile.argtypes = [ctypes.c_char_p]
            lib.axon_stop_nrt_profile.restype = ctypes.c_int64

            @contextlib.contextmanager
            def _hook(output_dir, device_ids):
                import jax
                jax.devices()
                if device_ids:
                    ids = (ctypes.c_int64 * len(device_ids))(*device_ids)
                    rc = lib.axon_start_nrt_profile(ids, len(device_ids))
                else:
                    rc = lib.axon_start_nrt_profile(None, 0)
                if rc != 0:
                    raise RuntimeError(f"axon_start_nrt_profile rc={rc}")
                try:
                    yield
                finally:
                    n = lib.axon_stop_nrt_profile(str(output_dir).encode())
                    if n < 0:
                        raise RuntimeError(f"axon_stop_nrt_profile rc={n}")
            return _hook
        except Exception:
            return None

    def get_axon_ntff_profile_hook():
        if 'h' not in _state:
            _state['h'] = _make_hook()
        return _state['h']

    mod.set_axon_ntff_profile_hook = set_axon_ntff_profile_hook
    mod.get_axon_ntff_profile_hook = get_axon_ntff_profile_hook
    sys.modules['antenv.axon_hooks'] = mod
    try:
        import antenv
        antenv.axon_hooks = mod
    except Exception:
        pass


_ensure_axon_hooks()

P = 128
NS = 32768
Q1 = 256
Q2 = 512
NCB = 16385
WIN = 512
NCO = 257
NF = 128
CT = [(0, 86), (86, 172), (172, 257)]
GP = 2            # pairs per group
GPB = 4           # groups per For_i body
PPB = GP * GPB    # pairs per body
NITER = 8         # bodies (npair = 64 per core)
DEBUG = False     # emit intermediate taps for group 0 (last body wins)


def _wm(n, m, denom, sign, scale=1.0):
    return np.exp(sign * 2j * np.pi * np.outer(np.arange(n), np.arange(m)) / denom) * scale


def build_consts():
    """fp16 weight/twiddle tables (dict c16) + fp32 tables (c32)."""
    c16, c32 = {}, {}
    s, si = -1, +1

    def put(d, name, z):
        d[name + '_r'] = z.real
        d[name + '_i'] = z.imag

    def twT(tw, nch, rep):
        """[k1, n] twiddle -> transposed chunks [128, nch*128] repeated rep times."""
        t = tw.T  # [n, k1]
        chunks = [t[ch * P:(ch + 1) * P, :] for ch in range(nch)]
        return np.concatenate(chunks * rep, axis=1)

    # ---- FFT_N forward ----
    put(c16, 'wpf', _wm(P, P, P, s, 1.0 / np.sqrt(NS)))
    put(c16, 'twfT', twT(_wm(P, Q1, NS, s), 2, GP))            # [128, 512]
    WQf = _wm(Q1, Q1, Q1, s)
    # merged r/i stage-2 weights: per chunk [Wr | Wi] and [-Wi | Wr]
    c16['wqf_ri'] = np.concatenate(
        [np.concatenate([WQf.real[ch * P:(ch + 1) * P], WQf.imag[ch * P:(ch + 1) * P]], 1)
         for ch in range(2)], 1)                                # [128, 1024]
    c16['wqf_nr'] = np.concatenate(
        [np.concatenate([-WQf.imag[ch * P:(ch + 1) * P], WQf.real[ch * P:(ch + 1) * P]], 1)
         for ch in range(2)], 1)
    # ---- FFT_N inverse ----
    WPi = _wm(P, P, P, si, 1.0 / np.sqrt(NS))
    put(c16, 'wpi', WPi)
    c16['wpi_in'] = -WPi.imag
    put(c16, 'twiT', twT(_wm(P, Q1, NS, si), 2, GP))
    WQi = _wm(Q1, Q1, Q1, si)
    c16['wqi_r'] = np.concatenate([WQi.real[0:P], WQi.real[P:2 * P]], 1)
    c16['wqi_in'] = -np.concatenate([WQi.imag[0:P], WQi.imag[P:2 * P]], 1)
    # ---- FFT_2N forward ----
    WPf2 = _wm(P, P, P, s, 1.0 / np.sqrt(2 * NS))
    c16['wpf2_r'], c16['wpf2_i'] = WPf2.real[0:64].copy(), WPf2.imag[0:64].copy()
    put(c16, 'twf2T', twT(_wm(P, Q2, 2 * NS, s), 4, 1))        # [128, 512]
    WQf2 = _wm(Q2, NCO, Q2, s)
    # merged r/i stage-2 weights over k2 in [0,256); Nyquist (k2=256) handled
    # by a separate side-path (only bin 32768 = row 0 is ever consumed).
    c16['wqf2_ri'] = np.concatenate(
        [np.concatenate([WQf2.real[ch * P:(ch + 1) * P, 0:256],
                         WQf2.imag[ch * P:(ch + 1) * P, 0:256]], 1)
         for ch in range(4)], 1)                                # [128, 2048]
    c16['wqf2_nr'] = np.concatenate(
        [np.concatenate([-WQf2.imag[ch * P:(ch + 1) * P, 0:256],
                         WQf2.real[ch * P:(ch + 1) * P, 0:256]], 1)
         for ch in range(4)], 1)
    # Nyquist helper: sum_j sum_c a2[j,c] * (-1)^c / sqrt(2N)
    alt = ((-1.0) ** np.arange(Q2)) / np.sqrt(2 * NS)
    c16['alt2n'] = np.broadcast_to(alt, (64, Q2)).copy()
    # ---- FFT_2N inverse ----
    WPi2 = _wm(P, P, P, si, 1.0 / np.sqrt(2 * NS))
    c16['wpi2_r'], c16['wpi2_i'] = WPi2.real[0:64].copy(), WPi2.imag[0:64].copy()
    c16['wpi2_in'] = -c16['wpi2_i']
    c16['wpi2h_r'], c16['wpi2h_i'] = WPi2.real[64:128].copy(), WPi2.imag[64:128].copy()
    c16['wpi2h_in'] = -c16['wpi2h_i']
    put(c16, 'twi2T', twT(_wm(P, Q2, 2 * NS, si), 4, 1))
    WQi2 = _wm(Q2, Q1, Q2, si)
    c16['wqi2_r'] = np.concatenate([WQi2.real[ch * P:(ch + 1) * P] for ch in range(4)], 1)
    c16['wqi2_in'] = -np.concatenate([WQi2.imag[ch * P:(ch + 1) * P] for ch in range(4)], 1)
    # ---- frame DFT ----
    w = np.arange(WIN)
    ham = 0.54 - 0.46 * np.cos(2.0 * np.pi * w / WIN)
    D = np.exp(-2j * np.pi * np.outer(w, np.arange(NCO)) / WIN) / np.sqrt(WIN)
    hamD = ham[:, None] * D
    c16['hd_r'] = np.concatenate([hamD.real[ch * P:(ch + 1) * P] for ch in range(4)], 1)
    c16['hd_i'] = np.concatenate([hamD.imag[ch * P:(ch + 1) * P] for ch in range(4)], 1)
    coef = np.ones(NCO)
    coef[1:256] = 2.0
    ang = 2.0 * np.pi * np.outer(np.arange(NCO), np.arange(WIN)) / WIN
    Er = (coef[:, None] * np.cos(ang)) / np.sqrt(WIN)
    Ei = (-(coef[:, None]) * np.sin(ang)) / np.sqrt(WIN)

    def echunk(E):
        out = np.zeros((86, 3 * Q2))
        for i, (lo, hi) in enumerate(CT):
            out[0:hi - lo, i * Q2:(i + 1) * Q2] = E[lo:hi]
        return out
    c16['e_r'], c16['e_i'] = echunk(Er), echunk(Ei)
    # ---- helpers ----
    eye = np.eye(P)
    c16['ident'] = eye
    c16['nident'] = -eye
    # jrevp col i selects input row 128-i (i>=1); col 0 zero (overwritten later).
    # Used for hermitian region B: out[j, i] = in[128-i, j] at 4B-aligned offsets.
    jrevp = np.roll(eye[::-1], 1, axis=0)
    jrevp[:, 0] = 0.0
    c16['jrevp'] = jrevp
    c16['njrevp'] = -jrevp
    t = np.arange(P)[:, None] + 128.0 * np.arange(Q1)[None, :]
    c32['tsq'] = t * t
    c32['ident32'] = eye
    c32['ones1'] = np.ones((1, P))
    c32['ones64'] = np.ones((64, 1))
    c16 = {k: np.ascontiguousarray(v, dtype=F16) for k, v in c16.items()}
    c32 = {k: np.ascontiguousarray(v, dtype=np.float32) for k, v in c32.items()}
    return c16, c32


def build_pair_data(x, noise):
    """Global per-pair host prep. Returns dict of [B*E, ...] arrays."""
    B, E = x.shape[:2]
    x = np.clip(x.astype(np.float64), 0.0, 1.0)
    means = x[..., 0]
    stds = x[..., 1]
    res = 0.01 + 0.99 * x[..., 2:259]
    spec_shape = x[..., 259:-1]
    amps = x[..., -1]
    npair = B * E

    d = {}
    d['noise'] = noise.reshape(npair, P, Q1).astype(F16)

    pos = np.clip((np.arange(NCB) + 0.5) * (128.0 / NCB) - 0.5, 0.0, 127.0)
    i0 = np.floor(pos).astype(int)
    i1 = np.minimum(i0 + 1, 127)
    wgt = pos - i0
    shp = spec_shape[..., i0] * (1.0 - wgt) + spec_shape[..., i1] * wgt
    full = np.zeros((B, E, NS))
    full[..., :NCB] = shp
    full[..., NCB:] = shp[..., 1:NCB - 1][..., ::-1]
    d['ginv'] = full.reshape(npair, P, Q1).astype(F16)

    sigma = np.clip((1e-8 + stds) * NS, 0.0, NS - 1.0)
    c1 = -0.5 / (sigma * sigma)
    corr = 1.0 / (1.0 + 1e-8 * sigma * np.sqrt(2.0 * np.pi))
    p3 = np.stack([np.repeat(c1[..., None], P, -1),
                   np.repeat(np.log(corr)[..., None], P, -1),
                   np.repeat(amps[..., None], P, -1)], axis=-1)  # [B,E,128,3]
    d['p3'] = p3.reshape(npair, P, 3).astype(np.float32)

    theta = 2.0 * np.pi * (means * 32768.0) / 32769.0
    u = np.exp(-1j * theta[..., None] * np.arange(P))
    v = np.exp(-1j * theta[..., None] * 128.0 * np.arange(NCO))
    uv = np.concatenate([
        np.stack([u.real, u.imag], axis=-2),                      # [B,E,2,128]
        np.stack([v.real, -v.imag], axis=-2),                     # [B,E,2,257]
        np.stack([v.imag, v.real], axis=-2)], axis=-1)            # [B,E,2,642]
    d['uv'] = uv.reshape(npair, 2, 642).astype(F16)

    r3 = np.zeros(res.shape[:-1] + (86, 3))
    r3[..., 0:86, 0] = res[..., 0:86]
    r3[..., 0:86, 1] = res[..., 86:172]
    r3[..., 0:85, 2] = res[..., 172:257]
    d['res3'] = r3.reshape(npair, 86, 3).astype(np.float32)
    return d


def _slab(a):
    """[64, X, Y] -> [NITER, X, PPB*Y] pair-major column slabs."""
    n, X, Y = a.shape
    assert n == NITER * PPB
    return np.ascontiguousarray(
        a.reshape(NITER, PPB, X, Y).transpose(0, 2, 1, 3).reshape(NITER, X, PPB * Y))


def build_program(n_batch, n_event):
    import concourse.bass as bass
    import concourse.mybir as mybir
    from concourse.tile import TileContext

    def split_excess_waits(nc_, max_waits=1):
        # this container's walrus rejects instructions with >2 sync waits;
        # hoist excess waits onto same-engine NoOps inserted before them.
        n_split = 0
        for f in nc_.m.functions:
            for bb in f.blocks:
                out = []
                for inst in bb.instructions:
                    si = inst.sync_info
                    waits = list(si.on_wait) if si is not None else []
                    if len(waits) > max_waits:
                        head, rest = waits[:max_waits], waits[max_waits:]
                        k = 0
                        while rest:
                            nop = mybir.InstNoOp(name=f"{inst.name}-w{k}", ins=[], outs=[])
                            nop.engine = inst.engine
                            nop.sync_info = mybir.SyncInfo(on_wait=rest[:max_waits],
                                                           on_update=[])
                            out.append(nop)
                            rest = rest[max_waits:]
                            k += 1
                        inst.sync_info = mybir.SyncInfo(on_wait=head,
                                                        on_update=list(si.on_update))
                        n_split += 1
                    out.append(inst)
                bb.instructions = out
        return n_split

    f32 = mybir.dt.float32
    f16 = mybir.dt.float16
    AT = mybir.ActivationFunctionType
    nc = bass.Bass()

    C16, C32 = build_consts()
    dt16 = {k: nc.dram_tensor(f"c_{k}", list(v.shape), f16, kind="ExternalInput")
            for k, v in C16.items()}
    dt32 = {k: nc.dram_tensor(f"c_{k}", list(v.shape), f32, kind="ExternalInput")
            for k, v in C32.items()}
    npair = n_batch * n_event
    assert npair == NITER * PPB
    din = {}
    din['noise'] = nc.dram_tensor("noise", [NITER, P, PPB * Q1], f16, kind="ExternalInput")
    din['ginv'] = nc.dram_tensor("ginv", [NITER, P, PPB * Q1], f16, kind="ExternalInput")
    din['p3'] = nc.dram_tensor("p3", [NITER, P, PPB * 3], f32, kind="ExternalInput")
    din['uv'] = nc.dram_tensor("uv", [NITER, 2, PPB * 642], f16, kind="ExternalInput")
    din['res3'] = nc.dram_tensor("res3", [NITER, 86, PPB * 3], f32, kind="ExternalInput")
    out_d = nc.dram_tensor("out", [n_batch, P, Q1], f32, kind="ExternalOutput")
    if DEBUG:
        dbg_d = nc.dram_tensor("dbg", [12, P, 1024], f16, kind="ExternalOutput")
        dbg32_d = nc.dram_tensor("dbg32", [1, P, Q2], f32, kind="ExternalOutput")

    with TileContext(nc) as tc:
        with tc.tile_pool(name="const", bufs=1) as cp, \
             tc.tile_pool(name="work", bufs=2) as wp, \
             tc.tile_pool(name="acc", bufs=1) as accp, \
             tc.tile_pool(name="ps", bufs=1, space="PSUM") as pp:
            ct = {}
            for k, v in C16.items():
                t = cp.tile(list(v.shape), f16, name=f"c_{k}", tag=f"c_{k}", bufs=1)
                nc.sync.dma_start(t[:], dt16[k][:])
                ct[k] = t
            for k, v in C32.items():
                t = cp.tile(list(v.shape), f32, name=f"c_{k}", tag=f"c_{k}", bufs=1)
                nc.sync.dma_start(t[:], dt32[k][:])
                ct[k] = t

            sig_tot = accp.tile([P, n_batch * Q1], f32, name="sigtot", tag="sigtot", bufs=1)
            nc.vector.memset(sig_tot[:], 0.0)

            def stg(name):
                return pp.tile([P, 1024], f32, name=name, tag="stg2", bufs=2)

            def stg3(name):
                return pp.tile([P, 2, Q2], f32, name=name, tag="stg2", bufs=2)

            def stg1(name):
                return pp.tile([P, 512], f32, name=name, tag="stg1", bufs=2)

            def tbank(name):
                return pp.tile([P, 1024], f16, name=name, tag="T", bufs=2)

            def wt(shape, name, tag=None, bufs=3):
                return wp.tile(shape, f16, name=name, tag=tag or name, bufs=bufs)

            def tr(out_psum, in_sbuf, iname='ident', ksize=None, ncols=None):
                idap = ct[iname]
                if ksize is not None:
                    idap = idap[0:ksize, 0:ksize]
                elif ncols is not None:
                    idap = idap[:, 0:ncols]
                else:
                    idap = idap[:]
                nc.tensor.transpose(out_psum, in_sbuf, idap)

            def cmulT(Tsrc, twname, out_r, out_i):
                """(out_r + i out_i) = Tsrc (r|i halves, PSUM fp16) * twiddle const."""
                u1 = wp.tile([P, 512], f16, name="u1", tag="u1", bufs=2)
                u2 = wp.tile([P, 512], f16, name="u2", tag="u2", bufs=2)
                twr = ct[twname + '_r'][:]
                twi = ct[twname + '_i'][:]
                nc.vector.tensor_mul(u1[:], Tsrc[:, 0:512], twr)
                nc.vector.tensor_mul(u2[:], Tsrc[:, 512:1024], twi)
                nc.vector.tensor_sub(out_r, u1[:], u2[:])
                u3 = wp.tile([P, 512], f16, name="u3", tag="u3", bufs=2)
                u4 = wp.tile([P, 512], f16, name="u4", tag="u4", bufs=2)
                nc.vector.tensor_mul(u3[:], Tsrc[:, 0:512], twi)
                nc.vector.tensor_mul(u4[:], Tsrc[:, 512:1024], twr)
                nc.vector.tensor_add(out_i, u3[:], u4[:])

            def dtap(slot, ap, r0=0, c0=0):
                if DEBUG:
                    pp_, ff_ = ap.partition_size(), ap.free_size()
                    nc.sync.dma_start(dbg_d[slot, r0:r0 + pp_, c0:c0 + ff_], ap)

            with tc.For_i(0, NITER, 1, name="body") as it:
                noise_s = wp.tile([P, PPB * Q1], f16, name="noise_s", tag="noise_s", bufs=1)
                ginv_s = wp.tile([P, PPB * Q1], f16, name="ginv_s", tag="ginv_s", bufs=1)
                p3_s = wp.tile([P, PPB * 3], f32, name="p3_s", tag="p3_s", bufs=1)
                uv_s = wp.tile([2, PPB * 642], f16, name="uv_s", tag="uv_s", bufs=1)
                res3_s = wp.tile([86, PPB * 3], f32, name="res3_s", tag="res3_s", bufs=1)
                nc.sync.dma_start(noise_s[:], din['noise'][it, :, :])
                nc.sync.dma_start(ginv_s[:], din['ginv'][it, :, :])
                nc.sync.dma_start(p3_s[:], din['p3'][it, :, :])
                nc.sync.dma_start(uv_s[:], din['uv'][it, :, :])
                nc.sync.dma_start(res3_s[:], din['res3'][it, :, :])

                body_acc = wp.tile([P, Q1], f32, name="body_acc", tag="body_acc", bufs=1)
                nc.vector.memset(body_acc[:], 0.0)

                def make_group(g):
                    """Return the ordered list of stage closures for group g."""
                    pr0 = GP * g
                    cols2 = slice(pr0 * Q1, (pr0 + GP) * Q1)
                    st = {}

                    def cmul2(srcR, srcI, twname, out_r, out_i):
                        """complex mul by twiddle const; all operands SBUF fp16."""
                        u1 = wp.tile([P, 512], f16, name="u1", tag="u1", bufs=2)
                        u2 = wp.tile([P, 512], f16, name="u2", tag="u2", bufs=2)
                        twr = ct[twname + '_r'][:]
                        twi = ct[twname + '_i'][:]
                        nc.vector.tensor_mul(u1[:], srcR, twr)
                        nc.vector.tensor_mul(u2[:], srcI, twi)
                        nc.vector.tensor_sub(out_r, u1[:], u2[:])
                        u3 = wp.tile([P, 512], f16, name="u3", tag="u3", bufs=2)
                        u4 = wp.tile([P, 512], f16, name="u4", tag="u4", bufs=2)
                        nc.vector.tensor_mul(u3[:], srcR, twi)
                        nc.vector.tensor_mul(u4[:], srcI, twr)
                        nc.vector.tensor_add(out_i, u3[:], u4[:])

                    def dmaT(dst, src):
                        """blocked 128x128 transposes: dst[p, b, c] = src[c, b*128+p]."""
                        eng = nc.sync if (st.setdefault('dq', 0) % 2 == 0) else nc.scalar
                        st['dq'] += 1
                        eng.dma_start_transpose(
                            dst.rearrange('p (b c) -> p b c', c=P), src)

                    def sA1():   # fwd FFT_N stage1 + drain
                        psAB = stg("psAB")
                        nc.tensor.matmul(psAB[:, 0:512], ct['wpf_r'][:], noise_s[:, cols2],
                                         start=True, stop=True)
                        nc.tensor.matmul(psAB[:, 512:1024], ct['wpf_i'][:], noise_s[:, cols2],
                                         start=True, stop=True)
                        aa = wt([P, 1024], "aa")
                        nc.scalar.copy(aa[:], psAB[:])
                        st['aa'] = aa

                    def sA2():   # transpose via DMA xbar + fused twiddle
                        TA = tbank("TA")
                        aa = st['aa']
                        for q in range(2):
                            for p in range(GP):
                                for ch in range(2):
                                    idx = q * 4 + p * 2 + ch
                                    tr(TA[:, idx * P:(idx + 1) * P],
                                       aa[:, q * 512 + p * Q1 + ch * P:
                                          q * 512 + p * Q1 + (ch + 1) * P])
                        btr = wt([P, 512], "btr")
                        bti = wt([P, 512], "bti")
                        cmul2(TA[:, 0:512], TA[:, 512:1024], 'twfT', btr[:], bti[:])
                        st['btr'], st['bti'] = btr, bti

                    def sA3():   # stage2 (merged r/i outputs) + drain
                        psCD = stg("psCD")
                        btr, bti = st['btr'], st['bti']
                        for p in range(GP):
                            for ch in range(2):
                                lr = btr[:, (p * 2 + ch) * P:(p * 2 + ch + 1) * P]
                                li = bti[:, (p * 2 + ch) * P:(p * 2 + ch + 1) * P]
                                nc.tensor.matmul(psCD[:, p * 512:(p + 1) * 512], lr,
                                                 ct['wqf_ri'][:, ch * 512:(ch + 1) * 512],
                                                 start=(ch == 0), stop=False)
                                nc.tensor.matmul(psCD[:, p * 512:(p + 1) * 512], li,
                                                 ct['wqf_nr'][:, ch * 512:(ch + 1) * 512],
                                                 start=False, stop=(ch == 1))
                        sp2 = wt([P, 1024], "sp2")   # per pair [Xr(256) | Xi(256)]
                        nc.scalar.copy(sp2[:], psCD[:])
                        st['sp2'] = sp2
                        if DEBUG and g == 0:
                            dtap(0, st['aa'][:])
                            dtap(1, sp2[:])

                    def sB():    # shape mult + inverse-grid relayout
                        sp2 = st['sp2']
                        TB = tbank("TB")
                        for q in range(2):
                            for p in range(GP):
                                for half in range(2):
                                    idx = q * 4 + p * 2 + half
                                    tr(TB[:, idx * P:(idx + 1) * P],
                                       sp2[:, p * 512 + q * Q1 + half:
                                           p * 512 + (q + 1) * Q1:2])
                        inv_r = wt([P, 512], "inv_r")
                        inv_i = wt([P, 512], "inv_i")
                        nc.vector.tensor_mul(inv_r[:], TB[:, 0:512], ginv_s[:, cols2])
                        nc.vector.tensor_mul(inv_i[:], TB[:, 512:1024], ginv_s[:, cols2])
                        st['inv_r'], st['inv_i'] = inv_r, inv_i
                        if DEBUG and g == 0:
                            dtap(2, inv_r[:])
                            dtap(3, inv_i[:])

                    def sC1():   # inverse FFT_N stage1 + drain
                        psAB2 = stg("psAB2")
                        nc.tensor.matmul(psAB2[:, 0:512], ct['wpi_r'][:], st['inv_r'][:],
                                         start=True, stop=False)
                        nc.tensor.matmul(psAB2[:, 0:512], ct['wpi_in'][:], st['inv_i'][:],
                                         start=False, stop=True)
                        nc.tensor.matmul(psAB2[:, 512:1024], ct['wpi_i'][:], st['inv_r'][:],
                                         start=True, stop=False)
                        nc.tensor.matmul(psAB2[:, 512:1024], ct['wpi_r'][:], st['inv_i'][:],
                                         start=False, stop=True)
                        cc = wt([P, 1024], "cc")
                        nc.scalar.copy(cc[:], psAB2[:])
                        st['cc'] = cc

                    def sC2():
                        TC = tbank("TC")
                        cc = st['cc']
                        for q in range(2):
                            for p in range(GP):
                                for ch in range(2):
                                    idx = q * 4 + p * 2 + ch
                                    tr(TC[:, idx * P:(idx + 1) * P],
                                       cc[:, q * 512 + p * Q1 + ch * P:
                                          q * 512 + p * Q1 + (ch + 1) * P])
                        ctr = wt([P, 512], "ctr")
                        cti = wt([P, 512], "cti")
                        cmul2(TC[:, 0:512], TC[:, 512:1024], 'twiT', ctr[:], cti[:])
                        st['ctr'], st['cti'] = ctr, cti

                    def sC3():   # stage2 (real out) + envelope
                        psE = stg1("psE")
                        ctr, cti = st['ctr'], st['cti']
                        for p in range(GP):
                            for ch in range(2):
                                lr = ctr[:, (p * 2 + ch) * P:(p * 2 + ch + 1) * P]
                                li = cti[:, (p * 2 + ch) * P:(p * 2 + ch + 1) * P]
                                nc.tensor.matmul(psE[:, p * Q1:(p + 1) * Q1], lr,
                                                 ct['wqi_r'][:, ch * Q1:(ch + 1) * Q1],
                                                 start=(ch == 0), stop=False)
                                nc.tensor.matmul(psE[:, p * Q1:(p + 1) * Q1], li,
                                                 ct['wqi_in'][:, ch * Q1:(ch + 1) * Q1],
                                                 start=False, stop=(ch == 1))
                        probs = wt([P, 512], "probs")
                        for p in range(GP):
                            pc = (pr0 + p) * 3
                            nc.scalar.activation(probs[:, p * Q1:(p + 1) * Q1], ct['tsq'][:],
                                                 AT.Exp, bias=p3_s[:, pc + 1:pc + 2],
                                                 scale=p3_s[:, pc:pc + 1])
                        a_y = wt([P, 512], "a_y")
                        nc.vector.tensor_mul(a_y[:], psE[:, 0:512], probs[:])
                        st['a_y'] = a_y
                        if DEBUG and g == 0:
                            dtap(4, a_y[:])

                    def sE():    # remap y-grid -> a2 (strided transposes stay on PE)
                        a_y = st['a_y']
                        TE = tbank("TE")
                        for p in range(GP):
                            for q in range(4):
                                idx = p * 4 + q
                                tr(TE[0:64, idx * P:(idx + 1) * P],
                                   a_y[:, p * Q1 + q:(p + 1) * Q1:4][:, 0:64])
                        a2 = wt([64, 1024], "a2")
                        nc.scalar.copy(a2[:], TE[0:64, :])
                        st['a2'] = a2
                        if DEBUG and g == 0:
                            dtap(5, a2[:])

                    def mk_sF(p):
                        def sF():   # fwd FFT_2N for pair p (merged stage2 + nyquist)
                            if p == 0:
                                s2 = wt([P, GP, 2, NCO], "s2", bufs=2)
                                nc.vector.memset(s2[:, :, :, 256:257], 0.0)
                                st['s2'] = s2
                            s2 = st['s2']
                            a2p = st['a2'][:, p * 512:(p + 1) * 512]
                            psF = stg("psF")
                            nc.tensor.matmul(psF[:, 0:512], ct['wpf2_r'][:], a2p,
                                             start=True, stop=True)
                            nc.tensor.matmul(psF[:, 512:1024], ct['wpf2_i'][:], a2p,
                                             start=True, stop=True)
                            dd = wt([P, 1024], "dd")
                            nc.scalar.copy(dd[:], psF[:])
                            TF = tbank("TF")
                            for q in range(2):
                                for ch in range(4):
                                    idx = q * 4 + ch
                                    tr(TF[:, idx * P:(idx + 1) * P],
                                       dd[:, q * 512 + ch * P:q * 512 + (ch + 1) * P])
                            dtr = wt([P, 512], "dtr")
                            dti = wt([P, 512], "dti")
                            cmul2(TF[:, 0:512], TF[:, 512:1024], 'twf2T', dtr[:], dti[:])
                            psF2 = stg1("psF2")
                            for ch in range(4):
                                nc.tensor.matmul(psF2[:, 0:512], dtr[:, ch * P:(ch + 1) * P],
                                                 ct['wqf2_ri'][:, ch * 512:(ch + 1) * 512],
                                                 start=(ch == 0), stop=False)
                                nc.tensor.matmul(psF2[:, 0:512], dti[:, ch * P:(ch + 1) * P],
                                                 ct['wqf2_nr'][:, ch * 512:(ch + 1) * 512],
                                                 start=False, stop=(ch == 3))
                            nc.scalar.copy(s2[:, p, :, 0:256],
                                           psF2[:, 0:512].rearrange('p (b c) -> p b c', c=256))
                            # Nyquist bin 32768 (only row 0 is consumed downstream)
                            nyt = wt([64, 512], "nyt")
                            nc.vector.tensor_mul(nyt[:], a2p, ct['alt2n'][:])
                            nyv = wp.tile([64, 1], f32, name="nyv", tag="nyv", bufs=2)
                            nc.vector.tensor_reduce(nyv[:], nyt[:], axis=mybir.AxisListType.X,
                                                    op=mybir.AluOpType.add)
                            psN = stg1("psN")
                            nc.tensor.matmul(psN[0:1, 0:1], nyv[:], ct['ones64'][:],
                                             start=True, stop=True)
                            nc.vector.tensor_copy(s2[0:1, p, 0, 256:257], psN[0:1, 0:1])
                        return sF

                    def sG():    # phase multiply
                        s2 = st['s2']
                        ph = wt([P, GP, 2, NCO], "ph", bufs=2)
                        for p in range(GP):
                            pu = (pr0 + p) * 642
                            psG = stg3("psG")
                            nc.tensor.matmul(psG[:, 0, 0:NCO], uv_s[0:2, pu:pu + P],
                                             uv_s[0:2, pu + P:pu + P + NCO],
                                             start=True, stop=True)
                            nc.tensor.matmul(psG[:, 1, 0:NCO], uv_s[0:2, pu:pu + P],
                                             uv_s[0:2, pu + P + NCO:pu + 642],
                                             start=True, stop=True)
                            nc.scalar.copy(ph[:, p, :, :], psG[:, :, 0:NCO])
                        y_r = wt([P, GP, NCO], "y_r")
                        y_i = wt([P, GP, NCO], "y_i")
                        ty1 = wt([P, GP, NCO], "ty1")
                        ty2 = wt([P, GP, NCO], "ty2")
                        nc.vector.tensor_mul(ty1[:], s2[:, :, 0, :], ph[:, :, 0, :])
                        nc.vector.tensor_mul(ty2[:], s2[:, :, 1, :], ph[:, :, 1, :])
                        nc.vector.tensor_sub(y_r[:], ty1[:], ty2[:])
                        nc.vector.tensor_mul(ty1[:], s2[:, :, 0, :], ph[:, :, 1, :])
                        nc.vector.tensor_mul(ty2[:], s2[:, :, 1, :], ph[:, :, 0, :])
                        nc.vector.tensor_add(y_i[:], ty1[:], ty2[:])
                        st['y_r'], st['y_i'] = y_r, y_i
                        if DEBUG and g == 0:
                            dtap(6, s2[:, :, :, 0:256])
                            dtap(7, ph[:, :, :, 0:256])
                            dtap(8, y_r[:, :, 0:256])
                            dtap(8, y_i[:, :, 0:256], c0=512)

                    def sHpre():
                        yrev_r = wt([P, GP, NCO], "yrev_r")
                        yrev_i = wt([P, GP, NCO], "yrev_i")
                        nc.vector.tensor_copy(yrev_r[:], st['y_r'][:, :, 256::-1])
                        nc.vector.tensor_copy(yrev_i[:], st['y_i'][:, :, 256::-1])
                        st['yrev_r'], st['yrev_i'] = yrev_r, yrev_i
                        st['inv2'] = wt([64, 2048], "inv2", bufs=2)
                        st['inv2h'] = wt([64, 2048], "inv2h", bufs=2)

                    def mk_sH(p):
                        def sH():   # hermitian extension for pair p
                            y_r, y_i = st['y_r'], st['y_i']
                            yrev_r, yrev_i = st['yrev_r'], st['yrev_i']
                            inv2, inv2h = st['inv2'], st['inv2h']
                            TH = tbank("TH")
                            for q4 in range(4):
                                tr(TH[0:64, q4 * P:(q4 + 1) * P],
                                   y_r[:, p, q4::4][:, 0:64])
                                tr(TH[0:64, 512 + q4 * P:512 + (q4 + 1) * P],
                                   y_i[:, p, q4::4][:, 0:64])
                            nc.scalar.copy(inv2[:, p * 1024:(p + 1) * 1024], TH[0:64, :])
                            # region B: fp16 transpose-mode ignores matrix signs ->
                            # negate at drain; fp16 PSUM writes are 4B-granular ->
                            # m=0 fixups live in a separate bank at even columns.
                            TH2 = tbank("TH2")
                            for gam in range(4):
                                qp = 3 - gam
                                tr(TH2[0:64, gam * P:(gam + 1) * P],
                                   yrev_r[:, p, 4 - qp::4][:, 0:64], iname='jrevp')
                                tr(TH2[0:64, 512 + gam * P:512 + (gam + 1) * P],
                                   yrev_i[:, p, 4 - qp::4][:, 0:64], iname='jrevp')
                            TH3 = tbank("TH3")
                            tr(TH3[0:64, 0:1], yrev_r[0:1, p, 0:256:4], ksize=1)
                            tr(TH3[0:64, 8:9], yrev_i[0:1, p, 0:256:4], ksize=1)
                            for gam in range(1, 4):
                                tr(TH3[0:64, 2 * gam:2 * gam + 1],
                                   yrev_r[0:1, p, gam::4][:, 0:64], ksize=1)
                                tr(TH3[0:64, 8 + 2 * gam:8 + 2 * gam + 1],
                                   yrev_i[0:1, p, gam::4][:, 0:64], ksize=1)
                            nc.scalar.copy(inv2h[:, p * 1024:p * 1024 + 512], TH2[0:64, 0:512])
                            nc.scalar.mul(inv2h[:, p * 1024 + 512:(p + 1) * 1024],
                                          TH2[0:64, 512:1024], -1.0)
                            nc.scalar.copy(inv2h[:, p * 1024:p * 1024 + 512:P], TH3[0:64, 0:8:2])
                            nc.scalar.mul(inv2h[:, p * 1024 + 512:(p + 1) * 1024:P],
                                          TH3[0:64, 8:16:2], -1.0)
                            nc.vector.tensor_copy(inv2h[0:1, p * 1024:p * 1024 + 1],
                                                  y_r[0:1, p, 256:257])
                            nc.vector.tensor_copy(inv2h[0:1, p * 1024 + 512:p * 1024 + 513],
                                                  y_i[0:1, p, 256:257])
                            if DEBUG and g == 0 and p == 1:
                                dtap(9, inv2[:, 0:1024])
                                dtap(10, inv2h[:, 0:1024])
                        return sH

                    def mk_sI(p):
                        def sI():   # inverse FFT_2N for pair p -> atoms
                            if p == 0:
                                atoms = wt([P, GP, 258], "atoms")
                                nc.vector.memset(atoms[:, :, 256:258], 0.0)
                                st['atoms'] = atoms
                            atoms = st['atoms']
                            inv2, inv2h = st['inv2'], st['inv2h']
                            psI = stg("psI")
                            i2r = inv2[:, p * 1024:p * 1024 + 512]
                            i2i = inv2[:, p * 1024 + 512:(p + 1) * 1024]
                            i2hr = inv2h[:, p * 1024:p * 1024 + 512]
                            i2hi = inv2h[:, p * 1024 + 512:(p + 1) * 1024]
                            nc.tensor.matmul(psI[:, 0:512], ct['wpi2_r'][:], i2r, start=True, stop=False)
                            nc.tensor.matmul(psI[:, 0:512], ct['wpi2h_r'][:], i2hr, start=False, stop=False)
                            nc.tensor.matmul(psI[:, 0:512], ct['wpi2_in'][:], i2i, start=False, stop=False)
                            nc.tensor.matmul(psI[:, 0:512], ct['wpi2h_in'][:], i2hi, start=False, stop=True)
                            nc.tensor.matmul(psI[:, 512:1024], ct['wpi2_i'][:], i2r, start=True, stop=False)
                            nc.tensor.matmul(psI[:, 512:1024], ct['wpi2h_i'][:], i2hr, start=False, stop=False)
                            nc.tensor.matmul(psI[:, 512:1024], ct['wpi2_r'][:], i2i, start=False, stop=False)
                            nc.tensor.matmul(psI[:, 512:1024], ct['wpi2h_r'][:], i2hi, start=False, stop=True)
                            ee = wt([P, 1024], "ee")
                            nc.scalar.copy(ee[:], psI[:])
                            TI = tbank("TI")
                            for q in range(2):
                                for ch in range(4):
                                    idx = q * 4 + ch
                                    tr(TI[:, idx * P:(idx + 1) * P],
                                       ee[:, q * 512 + ch * P:q * 512 + (ch + 1) * P])
                            etr = wt([P, 512], "etr")
                            eti = wt([P, 512], "eti")
                            cmul2(TI[:, 0:512], TI[:, 512:1024], 'twi2T', etr[:], eti[:])
                            psI2 = stg1("psI2")
                            for ch in range(4):
                                nc.tensor.matmul(psI2[:, 0:Q1], etr[:, ch * P:(ch + 1) * P],
                                                 ct['wqi2_r'][:, ch * Q1:(ch + 1) * Q1],
                                                 start=(ch == 0), stop=False)
                                nc.tensor.matmul(psI2[:, 0:Q1], eti[:, ch * P:(ch + 1) * P],
                                                 ct['wqi2_in'][:, ch * Q1:(ch + 1) * Q1],
                                                 start=False, stop=(ch == 3))
                            pc = (pr0 + p) * 3
                            nc.scalar.activation(atoms[:, p, 0:256], psI2[:, 0:Q1], AT.Copy,
                                                 scale=p3_s[:, pc + 2:pc + 3])
                        return sI

                    def sJ():    # frame DFT + resonance scan
                        atoms = st['atoms']
                        fins = []
                        for i, (lo, hi) in enumerate(CT):
                            nch = hi - lo
                            psJ = stg3("psJ")
                            for u in range(4):
                                rhs = atoms[:, :, u:min(u + 256, 258):2]
                                nc.tensor.matmul(psJ[0:nch, 0, 0:256],
                                                 ct['hd_r'][:, u * NCO + lo:u * NCO + hi], rhs,
                                                 start=(u == 0), stop=(u == 3))
                                nc.tensor.matmul(psJ[0:nch, 1, 0:256],
                                                 ct['hd_i'][:, u * NCO + lo:u * NCO + hi], rhs,
                                                 start=(u == 0), stop=(u == 3))
                            fin_r = wp.tile([86, GP, NF], f16, name=f"finr{i}", tag=f"finr{i}", bufs=2)
                            fin_i = wp.tile([86, GP, NF], f16, name=f"fini{i}", tag=f"fini{i}", bufs=2)
                            for p in range(GP):
                                rb = res3_s[0:nch, (pr0 + p) * 3 + i:(pr0 + p) * 3 + i + 1]
                                rbb = rb.to_broadcast([nch, NF])
                                nc.vector.tensor_tensor_scan(
                                    fin_r[0:nch, p, :], rbb, psJ[0:nch, 0, p * NF:(p + 1) * NF],
                                    initial=psJ[0:nch, 0, p * NF:p * NF + 1],
                                    op0=mybir.AluOpType.mult, op1=mybir.AluOpType.add)
                                nc.vector.tensor_tensor_scan(
                                    fin_i[0:nch, p, :], rbb, psJ[0:nch, 1, p * NF:(p + 1) * NF],
                                    initial=psJ[0:nch, 1, p * NF:p * NF + 1],
                                    op0=mybir.AluOpType.mult, op1=mybir.AluOpType.add)
                            fins.append((fin_r, fin_i, nch))
                        st['fins'] = fins

                    def sK():    # inverse frame DFT + overlap-add + accumulate
                        fins = st['fins']
                        sg = wp.tile([P, GP, Q1], f32, name="sg", tag="sg", bufs=2)
                        for u in range(4):
                            psK = pp.tile([P, 2, GP, NF], f32, name="psK", tag="stg1", bufs=2)
                            for i, (fin_r, fin_i, nch) in enumerate(fins):
                                ws = slice(i * Q2 + u * P, i * Q2 + (u + 1) * P)
                                nc.tensor.matmul(psK[:, 0, :, :], ct['e_r'][0:nch, ws],
                                                 fin_r[0:nch, :, :], start=(i == 0), stop=False)
                                nc.tensor.matmul(psK[:, 0, :, :], ct['e_i'][0:nch, ws],
                                                 fin_i[0:nch, :, :], start=False, stop=(i == 2))
                            if u < 2:
                                nc.scalar.copy(sg[:, :, u::2], psK[:, 0, :, :])
                            else:
                                nc.vector.tensor_add(sg[:, :, u::2], sg[:, :, u::2],
                                                     psK[:, 0, :, 0:127])
                        if DEBUG and g == 0:
                            dtap(11, st['atoms'][:, :, 0:256])
                            dtap(11, fins[0][0][:], c0=512)
                            dtap(11, fins[0][1][:], c0=768)
                            nc.sync.dma_start(dbg32_d[0, :, :], sg[:])
                        nc.vector.tensor_add(body_acc[:], body_acc[:], sg[:, 0, :])
                        nc.vector.tensor_add(body_acc[:], body_acc[:], sg[:, 1, :])

                    return [sA1, sA2, sA3, sB, sC1, sC2, sC3, sE,
                            mk_sF(0), mk_sF(1), sG, sHpre, mk_sH(0), mk_sH(1),
                            mk_sI(0), mk_sI(1), sJ, sK]

                # interleave pairs of groups stage-by-stage
                for g0 in range(0, GPB, 2):
                    for fs in zip(make_group(g0), make_group(g0 + 1)):
                        for f in fs:
                            f()

                sslice = sig_tot[:, bass.ds((it // 2) * Q1, Q1)]
                nc.vector.tensor_add(sslice, sig_tot[:, bass.ds((it // 2) * Q1, Q1)],
                                     body_acc[:])

            # ---------- max_norm per batch ----------
            for b in range(n_batch):
                sb_ = sig_tot[:, b * Q1:(b + 1) * Q1]
                mx = wp.tile([P, 1], f32, name="mx", tag="mx", bufs=1)
                nc.vector.tensor_reduce(mx[:], sb_, axis=mybir.AxisListType.X,
                                        op=mybir.AluOpType.max, apply_absolute_value=True)
                tpm = stg("tpm")
                nc.tensor.transpose(tpm[0:1, 0:P], mx[:], ct['ident32'][:])
                mxs = wp.tile([1, P], f32, name="mxs", tag="mxs", bufs=1)
                nc.scalar.copy(mxs[:], tpm[0:1, 0:P])
                m11 = wp.tile([1, 1], f32, name="m11", tag="m11", bufs=1)
                nc.vector.tensor_reduce(m11[:], mxs[:], axis=mybir.AxisListType.X,
                                        op=mybir.AluOpType.max)
                bc = stg("bc")
                nc.tensor.matmul(bc[:, 0:1], ct['ones1'][:], m11[:], start=True, stop=True)
                bcs = wp.tile([P, 1], f32, name="bcs", tag="bcs", bufs=1)
                nc.vector.tensor_scalar_add(bcs[:], bc[:, 0:1], 1e-8)
                rcp = wp.tile([P, 1], f32, name="rcp", tag="rcp", bufs=1)
                nc.vector.reciprocal(rcp[:], bcs[:])
                outt = wp.tile([P, Q1], f32, name="outt", tag="outt", bufs=1)
                nc.scalar.activation(outt[:], sb_, AT.Copy, scale=rcp[:])
                nc.sync.dma_start(out_d[b, :, :], outt[:])

    split_excess_waits(nc)
    return nc, C16, C32


def kernel(x, noise):
    from concourse.bass_utils import run_bass_kernel_spmd
    x = np.asarray(x, dtype=np.float32)
    noise = np.asarray(noise, dtype=np.float32)
    B, E = x.shape[:2]
    n_cores = 8
    nb = B // n_cores
    nc, C16, C32 = build_program(nb, E)
    pd = build_pair_data(x, noise)
    in_maps = []
    for c in range(n_cores):
        m = {f"c_{k}": v for k, v in C16.items()}
        m.update({f"c_{k}": v for k, v in C32.items()})
        sl = slice(c * nb * E, (c + 1) * nb * E)
        m['noise'] = _slab(pd['noise'][sl])
        m['ginv'] = _slab(pd['ginv'][sl])
        m['p3'] = _slab(pd['p3'][sl])
        m['uv'] = _slab(pd['uv'][sl])
        m['res3'] = _slab(pd['res3'][sl])
        in_maps.append(m)
    res = run_bass_kernel_spmd(nc, in_maps, core_ids=list(range(n_cores)))
    global LAST_RESULT
    LAST_RESULT = res
    out = np.zeros((B, 1, NS), dtype=np.float32)
    for c in range(n_cores):
        o = res.results[c]['out']
        for bb in range(nb):
            out[c * nb + bb, 0, :] = o[bb].T.reshape(-1)
    return out
